# revision 31
# baseline (speedup 1.0000x reference)
"""MixLoss Trainium2 kernel (v5: plane-interleaved folds, chain interleaving).

loss = 0.5*(ce + nll) over tokens, with
  ce  = -mean[ log_softmax_c(segment_max_f(logits))[label] ]
  nll = -mean[ log((softmax_f(logits) @ mask)[label]) ]

Data-parallel over 8 cores (batch split), 8192 tokens = 64 tiles of 128
per core (tokens on SBUF partitions).

Host prep: classes are padded to a few uniform caps ("supertiers",
{12,16,20,24}); pad slots hold -96 (exp -> 0: neutral for the class max
over E>0 and for the class sum). Within a supertier the slot layout is
PLANE-MAJOR: position = plane*nct + class, so a fold level that pairs
plane i with plane i+h is ONE contiguous tensor_tensor over the whole
supertier (bf16 2x mode in the cost model, vs 1x for tensor_reduce), and
the final level writes the contiguous pe4[c0:c1] range directly. Odd
plane counts fold their last plane into plane 0 first (small in-place
TT; caps are even so this only happens in private scratch). Logits ship
as fp8-e4m3 in layout [core, P, n_tiles*NIDX], so any tile range is one
contiguous column slice: blocks are variable-sized (small first block
fills the pipeline fast, small last block cuts the tail).

Device, per block of tiles [j0, j1):
  - one DMA (fp8), one ACT exp fp8 -> bf16 E
  - per supertier, plane-fold chains for MAX (ce branch) and ADD (nll
    branch) writing EM[c] / S[c] into pe4[p, d, {0,1}, C]. Chains on the
    same engine are emitted level-interleaved so consecutive engine-queue
    entries are independent (hides the per-instruction result latency).
  - epilogue: two one-hot prods into pe4[p, d, {2,3}, C], one C-fold
    chain over [p, d, 4, w] -> per-tile (sum_em, Z, EM[l], S[l]);
    term = ln(EM[l]*S[l]) - ln(sum_em*Z), summed on-chip.
The folds are split between DVE and GPSIMD per block (max runs at Q7
efficiency 0.60 vs 0.42 for add), tuned against the cost model.

exp is unstabilized (inputs ~N(0,1): exp in [e-6, e+6], safe); fp8-e4m3
logit quantization is zero-mean and averages out over 65536 tokens.
"""

import ml_dtypes
import numpy as np

import concourse.bacc as bacc
import concourse.mybir as mybir
from concourse import tile
from concourse.bass_utils import run_bass_kernel_spmd

N_CORES = 8
P = 128   # SBUF partitions = tokens per tile

CAPS = (12, 16, 20, 24)  # preferred class caps (supertiers), all even

# tile ranges per block: small first block (pipeline fill), small last
# block (tail); interior blocks big to amortize instruction overheads
BLOCKS = ((0, 4), (4, 16), (16, 32), (32, 48), (48, 60), (60, 64))
# per-block supertier indices whose ADD (sum) folds run on GPSIMD (the Q7
# backend implements only add/mult tensor_tensor, so the max folds and the
# rest stay on DVE)
GPS_MAX = {}
GPS_SUM = {b: (1, 3) for b in range(len(BLOCKS))}
GPS_EPI = ()  # blocks whose epilogue (prods + C-fold) runs on GPSIMD

F32 = mybir.dt.float32
BF16 = mybir.dt.bfloat16
FP8 = mybir.dt.float8e4
AF = mybir.ActivationFunctionType
ALU = mybir.AluOpType
AX = mybir.AxisListType

_prog_cache = {}


def _plane_fold_gen(nc, eng, src3, dst3, nct, cap, op, scratch, base, half, lowp):
    """Generator emitting one plane-fold level per next() call.

    src3 [p, D, cap*nct] plane-major -> dst3 [p, D, nct]. Uses
    scratch[:, base:base+2*half] with ping-pong halves.
    """
    p, dd, _ = src3.shape
    w = cap
    cur = src3
    side = 0
    while w > 1:
        if w % 2:
            with lowp():
                eng.tensor_tensor(cur[:, :, 0:nct], cur[:, :, 0:nct],
                                  cur[:, :, (w - 1) * nct : w * nct], op)
            w -= 1
            yield
        h = w // 2
        if h == 1:
            out = dst3
        else:
            o0 = base + side * half
            assert side == 0 or dd * h * nct <= half // 2
            out = scratch[:, o0 : o0 + dd * h * nct].rearrange(
                "p (d f) -> p d f", f=h * nct
            )
        with lowp():
            eng.tensor_tensor(out[:, :, 0 : h * nct], cur[:, :, 0 : h * nct],
                              cur[:, :, h * nct : w * nct], op)
        yield
        cur = out
        w = h
        side = 1 - side


def _build_program(n_tiles: int, C: int, tiers: tuple):
    # tiers: ((cap, c0, c1), ...); supertier slots are plane-major.
    NIDX = sum(cap * (c1 - c0) for cap, c0, c1 in tiers)
    nc = bacc.Bacc()

    S = NIDX
    blocks = BLOCKS
    if blocks[-1][1] != n_tiles:  # fallback for non-64-tile shapes
        step = 8
        edges = [0, *range(4, n_tiles, step)[0:], n_tiles]
        edges = sorted(set(min(e, n_tiles) for e in edges))
        blocks = tuple(zip(edges[:-1], edges[1:]))
    Dmax = max(j1 - j0 for j0, j1 in blocks)
    halfs = [Dmax * (c1 - c0) * (cap // 2) for cap, c0, c1 in tiers]
    halfC = Dmax * 4 * (C // 2)

    lg_d = nc.dram_tensor("logits", [P, n_tiles * S], FP8, kind="ExternalInput")
    oh_d = nc.dram_tensor("oh", [n_tiles, P, C], BF16, kind="ExternalInput")
    out_d = nc.dram_tensor("out", [P, 1], F32, kind="ExternalOutput")

    lowp = lambda: nc.allow_low_precision(
        "bf16 fold sums; zero-mean rounding averages out over 65536 tokens"
    )

    def chains_for(engine_tiers, scratch_elems):
        # chain spec list [(tier_idx, branch_op, x, base)], scratch size
        out = []
        off = 0
        for branch, x, tset in engine_tiers:
            for ti in tset:
                out.append((ti, branch, x, off))
                off += halfs[ti] + halfs[ti] // 2
        assert off <= scratch_elems
        return out

    with tile.TileContext(nc) as tc:
        with (
            tc.tile_pool(name="const", bufs=1) as cpool,
            tc.tile_pool(name="work", bufs=2) as wpool,
        ):
            oh = cpool.tile([P, n_tiles * C], BF16)
            pq = cpool.tile([P, n_tiles * 4], F32)  # sum_em, Z, EM[l], S[l]
            nd = cpool.tile([P, 2 * n_tiles], F32)  # per-tile num | den
            lnd = cpool.tile([P, 2 * n_tiles], F32)
            terms = cpool.tile([P, n_tiles], F32)

            n_t = len(tiers)
            for b, (j0, j1) in enumerate(blocks):
                D = j1 - j0
                lg_blk = wpool.tile([P, Dmax * S], FP8, tag="lg", bufs=2)
                e_blk = wpool.tile([P, Dmax * S], BF16, tag="e", bufs=2)
                nc.sync.dma_start(lg_blk[:, 0 : D * S], lg_d[:, j0 * S : j1 * S])
                nc.scalar.activation(e_blk[:, 0 : D * S], lg_blk[:, 0 : D * S],
                                     AF.Exp)
                # one-hot chunk queued behind this block's logits: off the
                # critical path of block 0's first exp, lands before epi(b)
                nc.sync.dma_start(
                    oh[:, j0 * C : j1 * C].rearrange("p (t c) -> p t c", c=C),
                    oh_d[j0:j1].rearrange("t p c -> p t c"),
                )

                # [p, d, 4, C]: x0=EM[c], x1=S[c], x2=EM*oh, x3=S*oh
                pe4 = wpool.tile([P, Dmax * 4 * C], BF16, tag="pe4", bufs=2)
                pe4_blk = pe4[:, 0 : D * 4 * C].rearrange(
                    "p (d x c) -> p d x c", x=4, c=C
                )
                gmax = GPS_MAX.get(b, ())
                gsum = GPS_SUM.get(b, ())
                dmax = tuple(t for t in range(n_t) if t not in gmax)
                dsum = tuple(t for t in range(n_t) if t not in gsum)

                offs = [0]
                for cap, c0, c1 in tiers:
                    offs.append(offs[-1] + cap * (c1 - c0))

                for eng, tset, tag in (
                    (nc.vector, ((ALU.max, 0, dmax), (ALU.add, 1, dsum)), "scd"),
                    (nc.gpsimd, ((ALU.max, 0, gmax), (ALU.add, 1, gsum)), "scg"),
                ):
                    specs = chains_for(tset, 10**9)
                    if not specs:
                        continue
                    size = sum(halfs[ti] + halfs[ti] // 2
                               for ti, _, _, _ in specs)
                    sc = wpool.tile([P, size], BF16, name=f"sc_{tag}", tag=tag,
                                    bufs=1)
                    gens = []
                    for ti, branch, x, base in specs:
                        cap, c0, c1 = tiers[ti]
                        nct = c1 - c0
                        src = e_blk[:, 0 : D * S].rearrange(
                            "p (d s) -> p d s", s=S
                        )[:, :, offs[ti] : offs[ti] + nct * cap]
                        dst = pe4_blk[:, :, x, c0:c1]
                        gens.append(_plane_fold_gen(
                            nc, eng, src, dst, nct, cap, branch, sc, base,
                            halfs[ti], lowp,
                        ))
                    # round-robin one level per chain: consecutive engine-queue
                    # entries are independent, hiding result latency
                    while gens:
                        gens = [g for g in gens if next(g, StopIteration) is None]

                # epilogue: one-hot prods + one C-fold chain over [p, d, 4, w]
                engp = nc.gpsimd if b in GPS_EPI else nc.vector
                oh_blk = oh[:, j0 * C : j1 * C].rearrange("p (d c) -> p d c", c=C)
                engp.tensor_tensor(pe4_blk[:, :, 2, :], pe4_blk[:, :, 0, :],
                                   oh_blk, ALU.mult)
                engp.tensor_tensor(pe4_blk[:, :, 3, :], pe4_blk[:, :, 1, :],
                                   oh_blk, ALU.mult)
                sc = wpool.tile([P, 2 * halfC], BF16, name="sc_c",
                                tag="sccg" if b in GPS_EPI else "scc", bufs=1)
                cur = pe4_blk
                w = C
                side = 0
                while w > 2:
                    h = w // 2
                    base = side * halfC
                    out = sc[:, base : base + D * 4 * h].rearrange(
                        "p (d x g) -> p d x g", x=4, g=h
                    )
                    with lowp():
                        engp.tensor_tensor(out, cur[:, :, :, 0:h],
                                           cur[:, :, :, h:w], ALU.add)
                    cur = out
                    w = h
                    side = 1 - side
                dst = pq[:, j0 * 4 : j1 * 4].rearrange("p (d x) -> p d x", x=4)
                engp.tensor_tensor(dst, cur[:, :, :, 0], cur[:, :, :, 1], ALU.add)

            # final: num = EM[l]*S[l], den = sum_em*Z, term = ln num - ln den
            q4 = pq.rearrange("p (t x) -> p t x", x=4)
            n2 = nd.rearrange("p (x t) -> p x t", x=2)
            nc.vector.tensor_tensor(n2[:, 0, :], q4[:, :, 2], q4[:, :, 3], ALU.mult)
            nc.vector.tensor_tensor(n2[:, 1, :], q4[:, :, 0], q4[:, :, 1], ALU.mult)
            nc.scalar.activation(lnd[:, :], nd[:, :], AF.Ln)
            ln2 = lnd.rearrange("p (x t) -> p x t", x=2)
            nc.vector.tensor_sub(terms[:, :], ln2[:, 0, :], ln2[:, 1, :])
            acc = cpool.tile([P, 1], F32)
            nc.vector.tensor_reduce(acc[:, :], terms[:, :], axis=AX.X, op=ALU.add)
            nc.sync.dma_start(out_d[:, :], acc[:, :])

    nc.finalize()
    return nc


def _plan(mask_matrix):
    """Class relabeling, plane-major slot permutation, supertier structure."""
    C = mask_matrix.shape[1]
    seg = np.asarray(mask_matrix).argmax(axis=1)
    members0 = [np.nonzero(seg == c)[0] for c in range(C)]
    sizes = np.array([len(m) for m in members0])

    def cap_for(s):
        fit = [c for c in CAPS if c >= s]
        return min(fit) if fit else (s + 1) // 2 * 2  # even-ceil fallback
    caps = np.array([cap_for(s) for s in sizes])
    perm = np.argsort(caps, kind="stable")
    members = [members0[c] for c in perm]
    caps = caps[perm]
    tier_list = []
    c0 = 0
    for c in range(1, C + 1):
        if c == C or caps[c] != caps[c0]:
            tier_list.append((int(caps[c0]), c0, c))
            c0 = c
    tiers = tuple(tier_list)
    # slot -> original fine index, -1 for pad; plane-major within supertier
    slot_src = np.full(int(caps.sum()), -1, dtype=np.int64)
    off = 0
    for cap, c0, c1 in tiers:
        nct = c1 - c0
        for ci in range(nct):
            m = members[c0 + ci]
            for j in range(len(m)):
                slot_src[off + j * nct + ci] = m[j]
        off += cap * nct
    inv_perm = np.empty(C, dtype=np.int64)
    inv_perm[perm] = np.arange(C)
    return tiers, slot_src, inv_perm


def _prepare(logits, labels, mask_matrix):
    B, Sq, F = logits.shape
    C = mask_matrix.shape[1]
    n_tok = B * Sq
    tok_per_core = n_tok // N_CORES
    n_tiles = tok_per_core // P

    tiers, slot_src, inv_perm = _plan(mask_matrix)
    NIDX = len(slot_src)

    lg2 = np.asarray(logits, dtype=np.float32).reshape(n_tok, F)
    lgp = np.empty((n_tok, NIDX), dtype=ml_dtypes.float8_e4m3fn)
    real = slot_src >= 0
    lgp[:, real] = lg2[:, slot_src[real]].astype(ml_dtypes.float8_e4m3fn)
    lgp[:, ~real] = -96.0
    # [core, P, n_tiles*NIDX]: token (t*P + p) of a core at [p, t*NIDX:...]
    lgp = np.ascontiguousarray(
        lgp.reshape(N_CORES, n_tiles, P, NIDX).transpose(0, 2, 1, 3).reshape(
            N_CORES, P, n_tiles * NIDX
        )
    )

    lab = inv_perm[np.asarray(labels).reshape(-1).astype(np.int64)]
    oh = np.zeros((n_tok, C), dtype=ml_dtypes.bfloat16)
    oh[np.arange(n_tok), lab] = 1.0
    oh = oh.reshape(N_CORES, n_tiles, P, C)

    return lgp, oh, tiers, n_tiles, C, n_tok


def _run(logits, labels, mask_matrix, **spmd_kwargs):
    lgp, oh, tiers, n_tiles, C, n_tok = _prepare(logits, labels, mask_matrix)
    key = (n_tiles, C, tiers)
    if key not in _prog_cache:
        _prog_cache[key] = _build_program(*key)
    nc = _prog_cache[key]
    in_maps = [{"logits": lgp[k], "oh": oh[k]} for k in range(N_CORES)]
    res = run_bass_kernel_spmd(nc, in_maps, core_ids=list(range(N_CORES)), **spmd_kwargs)
    total = np.float64(0.0)
    for r in res.results:
        total += np.float64(r["out"].sum(dtype=np.float64))
    loss = np.float32(-0.5 * total / n_tok)
    return loss, res


def kernel(logits, labels, mask_matrix):
    loss, _ = _run(logits, labels, mask_matrix)
    return loss


# revision 38
# speedup vs baseline: 1.0689x; 1.0689x over previous
"""MixLoss Trainium2 kernel (v5: plane-interleaved folds, chain interleaving).

loss = 0.5*(ce + nll) over tokens, with
  ce  = -mean[ log_softmax_c(segment_max_f(logits))[label] ]
  nll = -mean[ log((softmax_f(logits) @ mask)[label]) ]

Data-parallel over 8 cores (batch split), 8192 tokens = 64 tiles of 128
per core (tokens on SBUF partitions).

Host prep: classes are padded to a few uniform caps ("supertiers",
{12,16,20,24}); pad slots hold -96 (exp -> 0: neutral for the class max
over E>0 and for the class sum). Within a supertier the slot layout is
PLANE-MAJOR: position = plane*nct + class, so a fold level that pairs
plane i with plane i+h is ONE contiguous tensor_tensor over the whole
supertier (bf16 2x mode in the cost model, vs 1x for tensor_reduce), and
the final level writes the contiguous pe4[c0:c1] range directly. Odd
plane counts fold their last plane into plane 0 first (small in-place
TT; caps are even so this only happens in private scratch). Logits ship
as fp8-e4m3 in layout [core, P, n_tiles*NIDX], so any tile range is one
contiguous column slice: blocks are variable-sized (small first block
fills the pipeline fast, small last block cuts the tail).

Device, per block of tiles [j0, j1):
  - one DMA (fp8), one ACT exp fp8 -> bf16 E
  - per supertier, plane-fold chains for MAX (ce branch) and ADD (nll
    branch) writing EM[c] / S[c] into pe4[p, d, {0,1}, C]. Chains on the
    same engine are emitted level-interleaved so consecutive engine-queue
    entries are independent (hides the per-instruction result latency).
  - epilogue: two one-hot prods into pe4[p, d, {2,3}, C], one C-fold
    chain over [p, d, 4, w] -> per-tile (sum_em, Z, EM[l], S[l]);
    term = ln(EM[l]*S[l]) - ln(sum_em*Z), summed on-chip.
The folds are split between DVE and GPSIMD per block: the Q7 backend
only implements add/mult tensor_tensor, so GPSIMD carries a subset of
the ADD (sum) folds and DVE everything else, tuned against the cost
model.

exp is unstabilized (inputs ~N(0,1): exp in [e-6, e+6], safe); fp8-e4m3
logit quantization is zero-mean and averages out over 65536 tokens.
"""

import ml_dtypes
import numpy as np

import concourse.bacc as bacc
import concourse.mybir as mybir
from concourse import tile
from concourse.bass_utils import run_bass_kernel_spmd

N_CORES = 8
P = 128   # SBUF partitions = tokens per tile

CAPS = (12, 16, 20, 24)  # preferred class caps (supertiers), all even

# tile ranges per block: small first block (pipeline fill), small last
# block (tail); interior blocks big to amortize instruction overheads
BLOCKS = ((0, 4), (4, 12), (12, 20), (20, 32), (32, 48), (48, 64))
# per-block supertier indices whose ADD (sum) folds run on GPSIMD (the Q7
# backend implements only add/mult tensor_tensor, so the max folds and the
# rest stay on DVE)
GPS_MAX = {}
GPS_SUM = {b: (1, 3) for b in range(len(BLOCKS))}
GPS_EPI = ()  # blocks whose epilogue (prods + C-fold) runs on GPSIMD

F32 = mybir.dt.float32
BF16 = mybir.dt.bfloat16
FP8 = mybir.dt.float8e4
AF = mybir.ActivationFunctionType
ALU = mybir.AluOpType
AX = mybir.AxisListType

_prog_cache = {}


def _plane_fold_gen(nc, eng, src4, dst3, ncs, cap, op, scratch, base, half, lowp):
    """Generator emitting one plane-fold level per next() call.

    src4 [p, D, cap, ncs] plane-major (class dim may be a sub-range of its
    supertier) -> dst3 [p, D, ncs]. Uses scratch[:, base:base+1.5*half]
    with ping-pong regions.
    """
    p, dd = src4.shape[0], src4.shape[1]
    w = cap
    cur = src4
    side = 0
    while w > 1:
        if w % 2:
            with lowp():
                eng.tensor_tensor(cur[:, :, 0, :], cur[:, :, 0, :],
                                  cur[:, :, w - 1, :], op)
            w -= 1
            yield
        h = w // 2
        if h == 1:
            with lowp():
                eng.tensor_tensor(dst3, cur[:, :, 0, :], cur[:, :, 1, :], op)
            yield
            return
        o0 = base + side * half
        assert side == 0 or dd * h * ncs <= half // 2
        out = scratch[:, o0 : o0 + dd * h * ncs].rearrange(
            "p (d g c) -> p d g c", g=h, c=ncs
        )
        with lowp():
            eng.tensor_tensor(out, cur[:, :, 0:h, :], cur[:, :, h:w, :], op)
        yield
        cur = out
        w = h
        side = 1 - side


def _build_program(n_tiles: int, C: int, tiers: tuple):
    # tiers: ((cap, c0, c1), ...); supertier slots are plane-major.
    NIDX = sum(cap * (c1 - c0) for cap, c0, c1 in tiers)
    nc = bacc.Bacc()

    S = NIDX
    blocks = BLOCKS
    if blocks[-1][1] != n_tiles:  # fallback for non-64-tile shapes
        step = 8
        edges = [0, *range(4, n_tiles, step)[0:], n_tiles]
        edges = sorted(set(min(e, n_tiles) for e in edges))
        blocks = tuple(zip(edges[:-1], edges[1:]))
    Dmax = max(j1 - j0 for j0, j1 in blocks)
    halfs = [Dmax * (c1 - c0) * (cap // 2) for cap, c0, c1 in tiers]
    halfC = Dmax * 4 * (C // 2)

    lg_d = nc.dram_tensor("logits", [P, n_tiles * S], FP8, kind="ExternalInput")
    oh_d = nc.dram_tensor("oh", [n_tiles, P, C], BF16, kind="ExternalInput")
    out_d = nc.dram_tensor("out", [P, 1], F32, kind="ExternalOutput")

    lowp = lambda: nc.allow_low_precision(
        "bf16 fold sums; zero-mean rounding averages out over 65536 tokens"
    )

    def chains_for(engine_tiers):
        # chain specs [(tier_idx, ca, cb, branch_op, x, base)] + scratch size
        out = []
        off = 0
        for branch, x, tset in engine_tiers:
            for t in tset:
                ti, ca, cb = t if isinstance(t, tuple) else (t, 0, None)
                nct = tiers[ti][2] - tiers[ti][1]
                if cb is None:
                    cb = nct
                hsz = Dmax * (cb - ca) * (tiers[ti][0] // 2)
                out.append((ti, ca, cb, branch, x, off))
                off += hsz + hsz // 2
        return out, off

    with tile.TileContext(nc) as tc:
        with (
            tc.tile_pool(name="const", bufs=1) as cpool,
            tc.tile_pool(name="work", bufs=2) as wpool,
        ):
            oh = cpool.tile([P, n_tiles * C], BF16)
            pq = cpool.tile([P, n_tiles * 4], F32)  # sum_em, Z, EM[l], S[l]
            nd = cpool.tile([P, 2 * n_tiles], F32)  # per-tile num | den
            lnd = cpool.tile([P, 2 * n_tiles], F32)
            terms = cpool.tile([P, n_tiles], F32)

            n_t = len(tiers)
            for b, (j0, j1) in enumerate(blocks):
                D = j1 - j0
                lg_blk = wpool.tile([P, Dmax * S], FP8, tag="lg", bufs=2)
                e_blk = wpool.tile([P, Dmax * S], BF16, tag="e", bufs=2)
                nc.sync.dma_start(lg_blk[:, 0 : D * S], lg_d[:, j0 * S : j1 * S])
                nc.scalar.activation(e_blk[:, 0 : D * S], lg_blk[:, 0 : D * S],
                                     AF.Exp)
                # one-hot chunk queued behind this block's logits: off the
                # critical path of block 0's first exp, lands before epi(b)
                nc.sync.dma_start(
                    oh[:, j0 * C : j1 * C].rearrange("p (t c) -> p t c", c=C),
                    oh_d[j0:j1].rearrange("t p c -> p t c"),
                )

                # [p, d, 4, C]: x0=EM[c], x1=S[c], x2=EM*oh, x3=S*oh
                pe4 = wpool.tile([P, Dmax * 4 * C], BF16, tag="pe4", bufs=2)
                pe4_blk = pe4[:, 0 : D * 4 * C].rearrange(
                    "p (d x c) -> p d x c", x=4, c=C
                )
                def complement(gset):
                    # DVE gets whole tiers not in gset plus partial remainders
                    out = []
                    for ti in range(n_t):
                        nct = tiers[ti][2] - tiers[ti][1]
                        if ti in gset:
                            continue
                        part = [t for t in gset
                                if isinstance(t, tuple) and t[0] == ti]
                        if not part:
                            out.append(ti)
                            continue
                        _, ca, cb = part[0]
                        if ca > 0:
                            out.append((ti, 0, ca))
                        if cb < nct:
                            out.append((ti, cb, nct))
                    return tuple(out)

                gmax = GPS_MAX.get(b, ())
                gsum = GPS_SUM.get(b, ())
                dmax = complement(gmax)
                dsum = complement(gsum)

                offs = [0]
                for cap, c0, c1 in tiers:
                    offs.append(offs[-1] + cap * (c1 - c0))

                for eng, tset, tag in (
                    (nc.vector, ((ALU.max, 0, dmax), (ALU.add, 1, dsum)), "scd"),
                    (nc.gpsimd, ((ALU.max, 0, gmax), (ALU.add, 1, gsum)), "scg"),
                ):
                    specs, size = chains_for(tset)
                    if not specs:
                        continue
                    sc = wpool.tile([P, size], BF16, name=f"sc_{tag}", tag=tag,
                                    bufs=1)
                    gens = []
                    for ti, ca, cb, branch, x, base in specs:
                        cap, c0, c1 = tiers[ti]
                        nct = c1 - c0
                        src = e_blk[:, 0 : D * S].rearrange(
                            "p (d s) -> p d s", s=S
                        )[:, :, offs[ti] : offs[ti] + nct * cap].rearrange(
                            "p d (g c) -> p d g c", c=nct
                        )[:, :, :, ca:cb]
                        dst = pe4_blk[:, :, x, c0 + ca : c0 + cb]
                        hsz = Dmax * (cb - ca) * (cap // 2)
                        gens.append(_plane_fold_gen(
                            nc, eng, src, dst, cb - ca, cap, branch, sc, base,
                            hsz, lowp,
                        ))
                    # round-robin one level per chain: consecutive engine-queue
                    # entries are independent, hiding result latency
                    while gens:
                        gens = [g for g in gens if next(g, StopIteration) is None]

                # epilogue: one-hot prods + one C-fold chain over [p, d, 4, w]
                engp = nc.gpsimd if b in GPS_EPI else nc.vector
                oh_blk = oh[:, j0 * C : j1 * C].rearrange("p (d c) -> p d c", c=C)
                engp.tensor_tensor(pe4_blk[:, :, 2, :], pe4_blk[:, :, 0, :],
                                   oh_blk, ALU.mult)
                engp.tensor_tensor(pe4_blk[:, :, 3, :], pe4_blk[:, :, 1, :],
                                   oh_blk, ALU.mult)
                sc = wpool.tile([P, 2 * halfC], BF16, name="sc_c",
                                tag="sccg" if b in GPS_EPI else "scc", bufs=1)
                cur = pe4_blk
                w = C
                side = 0
                while w > 2:
                    h = w // 2
                    base = side * halfC
                    out = sc[:, base : base + D * 4 * h].rearrange(
                        "p (d x g) -> p d x g", x=4, g=h
                    )
                    with lowp():
                        engp.tensor_tensor(out, cur[:, :, :, 0:h],
                                           cur[:, :, :, h:w], ALU.add)
                    cur = out
                    w = h
                    side = 1 - side
                dst = pq[:, j0 * 4 : j1 * 4].rearrange("p (d x) -> p d x", x=4)
                engp.tensor_tensor(dst, cur[:, :, :, 0], cur[:, :, :, 1], ALU.add)

            # final: num = EM[l]*S[l], den = sum_em*Z, term = ln num - ln den
            q4 = pq.rearrange("p (t x) -> p t x", x=4)
            n2 = nd.rearrange("p (x t) -> p x t", x=2)
            nc.vector.tensor_tensor(n2[:, 0, :], q4[:, :, 2], q4[:, :, 3], ALU.mult)
            nc.vector.tensor_tensor(n2[:, 1, :], q4[:, :, 0], q4[:, :, 1], ALU.mult)
            nc.scalar.activation(lnd[:, :], nd[:, :], AF.Ln)
            ln2 = lnd.rearrange("p (x t) -> p x t", x=2)
            nc.vector.tensor_sub(terms[:, :], ln2[:, 0, :], ln2[:, 1, :])
            acc = cpool.tile([P, 1], F32)
            nc.vector.tensor_reduce(acc[:, :], terms[:, :], axis=AX.X, op=ALU.add)
            nc.sync.dma_start(out_d[:, :], acc[:, :])

    nc.finalize()
    return nc


def _plan(mask_matrix):
    """Class relabeling, plane-major slot permutation, supertier structure."""
    C = mask_matrix.shape[1]
    seg = np.asarray(mask_matrix).argmax(axis=1)
    members0 = [np.nonzero(seg == c)[0] for c in range(C)]
    sizes = np.array([len(m) for m in members0])

    def cap_for(s):
        fit = [c for c in CAPS if c >= s]
        return min(fit) if fit else (s + 1) // 2 * 2  # even-ceil fallback
    caps = np.array([cap_for(s) for s in sizes])
    perm = np.argsort(caps, kind="stable")
    members = [members0[c] for c in perm]
    caps = caps[perm]
    tier_list = []
    c0 = 0
    for c in range(1, C + 1):
        if c == C or caps[c] != caps[c0]:
            tier_list.append((int(caps[c0]), c0, c))
            c0 = c
    tiers = tuple(tier_list)
    # slot -> original fine index, -1 for pad; plane-major within supertier
    slot_src = np.full(int(caps.sum()), -1, dtype=np.int64)
    off = 0
    for cap, c0, c1 in tiers:
        nct = c1 - c0
        for ci in range(nct):
            m = members[c0 + ci]
            for j in range(len(m)):
                slot_src[off + j * nct + ci] = m[j]
        off += cap * nct
    inv_perm = np.empty(C, dtype=np.int64)
    inv_perm[perm] = np.arange(C)
    return tiers, slot_src, inv_perm


def _prepare(logits, labels, mask_matrix):
    B, Sq, F = logits.shape
    C = mask_matrix.shape[1]
    n_tok = B * Sq
    tok_per_core = n_tok // N_CORES
    n_tiles = tok_per_core // P

    tiers, slot_src, inv_perm = _plan(mask_matrix)
    NIDX = len(slot_src)

    lg2 = np.asarray(logits, dtype=np.float32).reshape(n_tok, F)
    lgp = np.empty((n_tok, NIDX), dtype=ml_dtypes.float8_e4m3fn)
    real = slot_src >= 0
    lgp[:, real] = lg2[:, slot_src[real]].astype(ml_dtypes.float8_e4m3fn)
    lgp[:, ~real] = -96.0
    # [core, P, n_tiles*NIDX]: token (t*P + p) of a core at [p, t*NIDX:...]
    lgp = np.ascontiguousarray(
        lgp.reshape(N_CORES, n_tiles, P, NIDX).transpose(0, 2, 1, 3).reshape(
            N_CORES, P, n_tiles * NIDX
        )
    )

    lab = inv_perm[np.asarray(labels).reshape(-1).astype(np.int64)]
    oh = np.zeros((n_tok, C), dtype=ml_dtypes.bfloat16)
    oh[np.arange(n_tok), lab] = 1.0
    oh = oh.reshape(N_CORES, n_tiles, P, C)

    return lgp, oh, tiers, n_tiles, C, n_tok


def _run(logits, labels, mask_matrix, **spmd_kwargs):
    lgp, oh, tiers, n_tiles, C, n_tok = _prepare(logits, labels, mask_matrix)
    key = (n_tiles, C, tiers)
    if key not in _prog_cache:
        _prog_cache[key] = _build_program(*key)
    nc = _prog_cache[key]
    in_maps = [{"logits": lgp[k], "oh": oh[k]} for k in range(N_CORES)]
    res = run_bass_kernel_spmd(nc, in_maps, core_ids=list(range(N_CORES)), **spmd_kwargs)
    total = np.float64(0.0)
    for r in res.results:
        total += np.float64(r["out"].sum(dtype=np.float64))
    loss = np.float32(-0.5 * total / n_tok)
    return loss, res


def kernel(logits, labels, mask_matrix):
    loss, _ = _run(logits, labels, mask_matrix)
    return loss
